# revision 1
# baseline (speedup 1.0000x reference)
"""MiMoV2 GQA attention (B=2, S=2048, HID=4096, 32 Q heads / 8 KV heads,
HD=128, VD=96, partial RoPE 64, causal) on 8 TRN2 NeuronCores.

Sharding: tensor-parallel over heads. Core c owns Q heads 4c..4c+3 and KV
head c (Wq/Wk/Wv column shards, Wo row shard). Activations replicated; the
row-parallel Wo partial outputs are summed on the host (the unshard step).

Per-core dataflow (all matmuls in fp32r = 1 cycle/row on the PE at N>=256):
  A) Transpose hidden tiles on the PE (contraction needs HID on partitions),
     project to Q^T/K^T [d, t] and V^T, apply RoPE via a rotation matmul +
     DVE muls, transpose V to [t, vd], stage QT/KT/V_ext in DRAM.
  B) Flash-style causal attention per (batch, q-head) with scores computed
     TRANSPOSED (S^T[j, i]) so softmax never reduces over partitions:
     exp(scale*s) on ACT, block-skipping of the strictly-upper triangle,
     0/1 diagonal mask tiles, and the softmax denominator obtained free via
     a ones-column appended to V.  P^T @ V accumulates out^T [vd, i]; the
     per-query normalizer is broadcast with a K=1 matmul and applied on DVE.
  C) out = A @ Wo_shard with A^T staged [384, t]: 3 K-chunks accumulate in
     PSUM per [128 t, 512 c] tile.
"""
import numpy as np

import concourse.bacc as bacc
import concourse.mybir as mybir
import concourse.tile as tile

F32 = mybir.dt.float32
MM_DT = mybir.dt.float32r  # matmul operand dtype; float32r = fast, ~1e-4 rel

B, S, HID = 2, 2048, 4096
NH, NKV, HD, VD = 32, 8, 128, 96
ROPE = 64
NCORES = 8
QH = NH // NCORES            # 4 q heads per core
T = B * S                    # 4096 tokens
OSH = QH * VD                # 384 output dims per core
CO = NH * VD                 # 3072 full output dim
THETA = 1000000.0
SCALE = float(HD ** -0.5)

AFT = mybir.ActivationFunctionType


def _build():
    nc = bacc.Bacc("TRN2", target_bir_lowering=False, debug=False,
                   num_devices=NCORES)
    hidden = nc.declare_dram_parameter("hidden", [T, HID], MM_DT, False)
    wq = nc.declare_dram_parameter("wq", [HID, QH * HD], MM_DT, False)
    wk = nc.declare_dram_parameter("wk", [HID, HD], MM_DT, False)
    wv = nc.declare_dram_parameter("wv", [HID, VD], MM_DT, False)
    wo = nc.declare_dram_parameter("wo", [OSH, CO], MM_DT, False)
    cos = nc.declare_dram_parameter("cos", [ROPE, T], F32, False)
    sin = nc.declare_dram_parameter("sin", [ROPE, T], F32, False)
    masks = nc.declare_dram_parameter("masks", [4, 128, 512], F32, False)
    ident = nc.declare_dram_parameter("ident", [128, 128], MM_DT, False)
    out = nc.declare_dram_parameter("out", [CO, T], F32, True)

    qt_stage = [nc.dram_tensor(f"qt_stage{b}", [QH, HD, S], MM_DT)
                for b in range(B)]
    kt_stage = [nc.dram_tensor(f"kt_stage{b}", [HD, S], MM_DT)
                for b in range(B)]
    v_stage = [nc.dram_tensor(f"v_stage{b}", [128, (S // 128) * (VD + 1)],
                              MM_DT) for b in range(B)]
    at_stage = nc.dram_tensor("at_stage", [OSH, T], MM_DT)

    with tile.TileContext(nc) as tc:
        with tc.tile_pool(name="const", bufs=1) as cst:
            id_sb = cst.tile([128, 128], MM_DT, tag="ident")
            nc.sync.dma_start(out=id_sb[:], in_=ident[:])
            onesc_f = cst.tile([128, 1], F32, tag="onesc_f")
            nc.gpsimd.memset(onesc_f[:], 1.0)
            onesc = cst.tile([128, 1], MM_DT, tag="onesc")
            nc.vector.tensor_copy(onesc[:], onesc_f[:])
            ones96_f = cst.tile([1, 96], F32, tag="ones96_f")
            nc.gpsimd.memset(ones96_f[:], 1.0)
            ones96 = cst.tile([1, 96], MM_DT, tag="ones96")
            nc.vector.tensor_copy(ones96[:], ones96_f[:])

            # ---------------- Phase A: transpose + QKV + RoPE ----------------
            with (
                tc.tile_pool(name="wpool", bufs=1) as wpool,
                tc.tile_pool(name="apool", bufs=8) as apool,
                tc.tile_pool(name="htpool", bufs=4) as htpool,
                tc.tile_pool(name="qspool", bufs=3) as qspool,
                tc.tile_pool(name="accps", bufs=1, space="PSUM") as accps,
                tc.tile_pool(name="trps", bufs=2, space="PSUM") as trps,
            ):
                cos_sb = wpool.tile([ROPE, T], F32, tag="cos")
                nc.sync.dma_start(out=cos_sb[:], in_=cos[:])
                sin_sb = wpool.tile([ROPE, T], F32, tag="sin")
                nc.sync.dma_start(out=sin_sb[:], in_=sin[:])

                wq_r = wq.rearrange("(c p) o -> p c o", p=128)
                wk_r = wk.rearrange("(c p) o -> p c o", p=128)
                wv_r = wv.rearrange("(c p) o -> p c o", p=128)
                wq_sb = [[None] * 8 for _ in range(QH)]
                wk_sb = [None] * 8
                wv_sb = [None] * 8

                def load_weight_chunk(g):       # chunk of 4 h-blocks
                    gsl = slice(g * 4, (g + 1) * 4)
                    for h in range(QH):
                        wt = wpool.tile([128, 4, 128], MM_DT,
                                        tag=f"wq{h}_{g}", name=f"wq{h}_{g}")
                        nc.sync.dma_start(
                            out=wt[:], in_=wq_r[:, gsl, h * 128:(h + 1) * 128])
                        wq_sb[h][g] = wt
                    wkt = wpool.tile([128, 4, HD], MM_DT, tag=f"wk_{g}",
                                     name=f"wk_{g}")
                    nc.sync.dma_start(out=wkt[:], in_=wk_r[:, gsl, :])
                    wk_sb[g] = wkt
                    wvt = wpool.tile([128, 4, VD], MM_DT, tag=f"wv_{g}",
                                     name=f"wv_{g}")
                    nc.sync.dma_start(out=wvt[:], in_=wv_r[:, gsl, :])
                    wv_sb[g] = wvt

                def rope_finish(qs, dst2d, stsl, tsl):
                    half = ROPE // 2
                    t1 = qspool.tile([ROPE, 512], F32, tag="t1")
                    # t1[0:32] = -q[32:64]*sin ; t1[32:64] = q[0:32]*sin
                    nc.vector.tensor_mul(t1[0:half, :], qs[half:ROPE, :],
                                         sin_sb[half:ROPE, tsl])
                    nc.vector.tensor_mul(t1[half:ROPE, :], qs[0:half, :],
                                         sin_sb[0:half, tsl])
                    qcos = qspool.tile([ROPE, 512], F32, tag="qcos")
                    nc.vector.tensor_mul(qcos[:], qs[0:ROPE, :],
                                         cos_sb[:, tsl])
                    nc.vector.tensor_add(qs[0:ROPE, :], qcos[:], t1[:])
                    nc.gpsimd.dma_start(out=dst2d[:, stsl], in_=qs[:])

                pending_rope = []
                wchunks_loaded = 0
                for tt in range(T // 512):
                    b_, stsl = tt // 4, slice((tt % 4) * 512,
                                              (tt % 4) * 512 + 512)
                    tsl = slice(tt * 512, (tt + 1) * 512)
                    qacc = [accps.tile([128, 512], F32, tag=f"qacc{h}",
                                       name=f"qacc{h}_{tt}")
                            for h in range(QH)]
                    kacc = accps.tile([128, 512], F32, tag="kacc")
                    vacc = accps.tile([VD, 512], F32, tag="vacc")
                    for hg in range(HID // 512):
                        loads = []
                        for sub in range(4):
                            ld = apool.tile([128, 512], MM_DT, tag="load")
                            nc.sync.dma_start(
                                out=ld[:],
                                in_=hidden[tt * 512 + sub * 128:
                                           tt * 512 + (sub + 1) * 128,
                                           hg * 512:(hg + 1) * 512])
                            loads.append(ld)
                        while wchunks_loaded < min(8, hg + 3):
                            load_weight_chunk(wchunks_loaded)
                            wchunks_loaded += 1
                        for hc4 in range(4):
                            hc = hg * 4 + hc4
                            trp = trps.tile([128, 512], MM_DT, tag="tr")
                            for sub in range(4):
                                nc.tensor.matmul(
                                    trp[:, sub * 128:(sub + 1) * 128],
                                    loads[sub][:, hc4 * 128:(hc4 + 1) * 128],
                                    id_sb[:], start=(sub == 0),
                                    stop=(sub == 3), is_transpose=True)
                            ht = htpool.tile([128, 512], MM_DT, tag="ht")
                            nc.vector.tensor_copy(ht[:], trp[:])
                            st_, sp_ = hc == 0, hc == HID // 128 - 1
                            for h in range(QH):
                                nc.tensor.matmul(qacc[h][:],
                                                 wq_sb[h][hc // 4][:, hc % 4, :],
                                                 ht[:], start=st_, stop=sp_)
                            nc.tensor.matmul(kacc[:],
                                             wk_sb[hc // 4][:, hc % 4, :],
                                             ht[:], start=st_, stop=sp_)
                            nc.tensor.matmul(vacc[:],
                                             wv_sb[hc // 4][:, hc % 4, :],
                                             ht[:], start=st_, stop=sp_)
                        # trailing RoPE of the previous tile rides along here
                        if pending_rope:
                            rope_finish(*pending_rope.pop(0))
                    # -- boundary: free PSUM banks with short DVE casts first
                    vts = qspool.tile([VD, 512], MM_DT, tag="vts")
                    nc.vector.tensor_copy(vts[:], vacc[:])
                    qss = []
                    for h in range(QH):
                        qs = qspool.tile([128, 512], MM_DT, tag="qs",
                                         name=f"qs{h}_{tt}", bufs=8)
                        nc.vector.tensor_copy(qs[:], qacc[h][:])
                        qss.append(qs)
                    ks = qspool.tile([128, 512], MM_DT, tag="qs",
                                     name=f"ks_{tt}", bufs=8)
                    nc.vector.tensor_copy(ks[:], kacc[:])
                    # V transpose path (PE only waits for the vts cast)
                    for sub in range(4):
                        vtp = trps.tile([128, VD], MM_DT, tag="tr",
                                        name=f"vtp_{tt}_{sub}")
                        nc.tensor.matmul(vtp[:],
                                         vts[:, sub * 128:(sub + 1) * 128],
                                         id_sb[0:VD, 0:VD], start=True,
                                         stop=True, is_transpose=True)
                        vsb = qspool.tile([128, VD + 1], MM_DT, tag="vsb")
                        nc.vector.tensor_copy(vsb[:, 0:VD], vtp[:])
                        nc.vector.tensor_copy(vsb[:, VD:VD + 1], onesc[:])
                        ti = (tt % 4) * 4 + sub
                        nc.gpsimd.dma_start(
                            out=v_stage[b_][:, ti * (VD + 1):
                                            (ti + 1) * (VD + 1)],
                            in_=vsb[:])
                    # K first so phase B's KT load unblocks earliest
                    pending_rope = [(ks, kt_stage[b_], stsl, tsl)]
                    pending_rope += [(qss[h], qt_stage[b_][h], stsl, tsl)
                                     for h in range(QH)]
                for item in pending_rope:
                    rope_finish(*item)

            # ---------------- Phase B + C share the prefetched Wo ----------
            cpool_cm = tc.tile_pool(name="cpool", bufs=1)
            cpool = cpool_cm.__enter__()
            # ---------------- Phase B: causal attention ----------------
            with (
                tc.tile_pool(name="bpool", bufs=2) as bpool,
                tc.tile_pool(name="qtpool", bufs=3) as qtpool,
                tc.tile_pool(name="ptpool", bufs=4) as ptpool,
                tc.tile_pool(name="atpool", bufs=3) as atpool,
                tc.tile_pool(name="smpool", bufs=3) as smpool,
                tc.tile_pool(name="stps", bufs=3, space="PSUM") as stps,
                tc.tile_pool(name="accb", bufs=2, space="PSUM") as accb,
                tc.tile_pool(name="bcps", bufs=3, space="PSUM") as bcps,
            ):
                wo_sb = cpool.tile([128, OSH // 128, CO], MM_DT, tag="wo")
                nc.sync.dma_start(out=wo_sb[:],
                                  in_=wo.rearrange("(c p) o -> p c o", p=128))
                msk_sb = bpool.tile([128, 4, 512], F32, tag="masks")
                nc.sync.dma_start(out=msk_sb[:],
                                  in_=masks.rearrange("m p f -> p m f"))

                def emit_norm(p):
                    qh_, atus_, recr_, isls_ = p
                    for ib in range(S // 512):
                        bcp = bcps.tile([VD, 512], F32, tag="bc",
                                        name=f"bcp_{qh_}_{ib}_{isls_[ib].start}")
                        nc.tensor.matmul(bcp[:], ones96[:],
                                         recr_[ib][:],
                                         start=True, stop=True)
                        bcs = smpool.tile([VD, 512], F32, tag="bcs",
                                          name=f"bcs_{qh_}_{ib}_{isls_[ib].start}",
                                          bufs=2)
                        nc.vector.tensor_copy(bcs[:], bcp[:])
                        at = atpool.tile([VD, 512], MM_DT, tag="at",
                                         name=f"at_{qh_}_{ib}_{isls_[ib].start}")
                        nc.vector.tensor_mul(at[:], atus_[ib][:], bcs[:])
                        nc.gpsimd.dma_start(
                            out=at_stage[qh_ * VD:(qh_ + 1) * VD, isls_[ib]],
                            in_=at[:])

                pending = None
                for b in range(B):
                    ktb = bpool.tile([128, S, ], MM_DT, tag="ktb")
                    nc.sync.dma_start(out=ktb[:], in_=kt_stage[b][:, :])
                    vb = bpool.tile([128, (S // 128) * (VD + 1)], MM_DT,
                                    tag="vb")
                    nc.sync.dma_start(out=vb[:], in_=v_stage[b][:, :])
                    for qh in range(QH):
                        if pending is not None:
                            emit_norm(pending)
                            pending = None
                        l4 = smpool.tile([97, 512], F32, tag="l4",
                                         name=f"l4_{b}_{qh}", bufs=2)
                        atus = []
                        isls = []
                        for ib in range(S // 512):
                            isl = slice(b * S + ib * 512, b * S + (ib + 1) * 512)
                            isls.append(isl)
                            qt = qtpool.tile([128, 512], MM_DT, tag="qt")
                            nc.sync.dma_start(
                                out=qt[:],
                                in_=qt_stage[b][qh][:, ib * 512:(ib + 1) * 512])
                            acc = accb.tile([VD + 1, 512], F32, tag="acc")
                            nj = 4 * ib + 4
                            for jb in range(nj):
                                stp = stps.tile([128, 512], F32, tag="st")
                                nc.tensor.matmul(
                                    stp[:], ktb[:, jb * 128:(jb + 1) * 128],
                                    qt[:], start=True, stop=True)
                                st_, sp_ = (jb == 0), (jb == nj - 1)
                                if jb < 4 * ib:
                                    pt = ptpool.tile([128, 512], MM_DT,
                                                     tag="pt")
                                    nc.scalar.activation(pt[:], stp[:],
                                                         AFT.Exp, scale=SCALE)
                                    nc.tensor.matmul(
                                        acc[:],
                                        vb[:, jb * (VD + 1):(jb + 1) * (VD + 1)],
                                        pt[:], start=st_, stop=sp_)
                                else:
                                    # diagonal block: columns < r fully masked
                                    r = (jb - 4 * ib) * 128
                                    pt = ptpool.tile([128, 512], MM_DT,
                                                     tag="pt")
                                    nc.scalar.activation(pt[:, r:512],
                                                         stp[:, r:512],
                                                         AFT.Exp, scale=SCALE)
                                    ptm = ptpool.tile([128, 512], MM_DT,
                                                      tag="ptm")
                                    nc.vector.tensor_mul(
                                        ptm[:, r:512], pt[:, r:512],
                                        msk_sb[:, jb - 4 * ib, r:512])
                                    nc.tensor.matmul(
                                        acc[:, r:512],
                                        vb[:, jb * (VD + 1):(jb + 1) * (VD + 1)],
                                        ptm[:, r:512], start=st_, stop=sp_)
                            atu = atpool.tile([VD, 512], F32, tag="atu",
                                              name=f"atu_{b}_{qh}_{ib}",
                                              bufs=10)
                            nc.vector.tensor_copy(atu[:], acc[0:VD, :])
                            nc.vector.tensor_copy(l4[32 * ib:32 * ib + 1, :],
                                                  acc[VD:VD + 1, :])
                            atus.append(atu)
                        recf = smpool.tile([97, 512], F32, tag="lnt",
                                           name=f"recf_{b}_{qh}", bufs=2)
                        nc.vector.reciprocal(recf[:], l4[:])
                        recr = []
                        for ib in range(S // 512):
                            rr = smpool.tile([1, 512], MM_DT, tag=f"recr{ib}",
                                             name=f"recr_{b}_{qh}_{ib}",
                                             bufs=2)
                            nc.vector.tensor_copy(
                                rr[:], recf[32 * ib:32 * ib + 1, :])
                            recr.append(rr)
                        pending = (qh, atus, recr, isls)
                emit_norm(pending)
            # ---------------- Phase C: output projection (transposed) -------
            with (
                tc.tile_pool(name="cpool2", bufs=1) as cpool2,
                tc.tile_pool(name="ospool", bufs=6) as ospool,
                tc.tile_pool(name="cps", bufs=1, space="PSUM") as cps,
            ):
                at_sb = cpool2.tile([128, OSH // 128, T], MM_DT, tag="atall")
                at_r2 = at_stage.rearrange("(c p) t -> p c t", p=128)
                for oc in range(OSH // 128):
                    nc.sync.dma_start(out=at_sb[:, oc, :],
                                      in_=at_r2[:, oc, :])
                for ct in range(CO // 128):
                    ops = [cps.tile([128, 512], F32, tag=f"c{tb}",
                                    name=f"ops{ct}_{tb}")
                           for tb in range(T // 512)]
                    for oc in range(OSH // 128):
                        for tb in range(T // 512):
                            nc.tensor.matmul(
                                ops[tb][:],
                                wo_sb[:, oc, ct * 128:(ct + 1) * 128],
                                at_sb[:, oc, tb * 512:(tb + 1) * 512],
                                start=(oc == 0), stop=(oc == OSH // 128 - 1))
                    for tb in range(T // 512):
                        osb = ospool.tile([128, 512], F32, tag="osb")
                        if tb in (1, 4, 7):
                            nc.scalar.copy(osb[:], ops[tb][:])
                        else:
                            nc.vector.tensor_copy(osb[:], ops[tb][:])
                        nc.gpsimd.dma_start(
                            out=out[ct * 128:(ct + 1) * 128,
                                    tb * 512:(tb + 1) * 512],
                            in_=osb[:])
            cpool_cm.__exit__(None, None, None)

    nc.compile()
    return nc


_NC_CACHE = None


def _get_nc():
    global _NC_CACHE
    if _NC_CACHE is None:
        _NC_CACHE = _build()
    return _NC_CACHE


def _host_tables(position_ids):
    pos = np.asarray(position_ids).reshape(-1)[:S].astype(np.float64)
    inv_freq = 1.0 / (THETA ** (np.arange(0, ROPE, 2, dtype=np.float64) / ROPE))
    freqs = np.outer(pos, inv_freq)                       # [S, ROPE/2]
    emb = np.concatenate([freqs, freqs], axis=-1)         # [S, ROPE]
    cos_t = np.tile(np.cos(emb).astype(np.float32).T, (1, B))  # [ROPE, T]
    sinf = np.sin(freqs).astype(np.float32).T                  # [ROPE/2, S]
    # sinsh rows 0:32 = +sin (used for t1[32:64] = q[0:32]*sin)
    # sinsh rows 32:64 = -sin (used for t1[0:32] = -q[32:64]*sin)
    sin_t = np.tile(np.concatenate([sinf, -sinf], axis=0), (1, B))
    return cos_t, sin_t


def _host_masks(attention_mask):
    m = np.asarray(attention_mask).reshape(S, S)
    tiles = np.empty((4, 128, 512), dtype=np.float32)
    for r4 in range(4):
        r = r4 * 128
        tiles[r4] = (m[0:512, r:r + 128] == 0.0).T.astype(np.float32)
    return tiles


def kernel(hidden_states, attention_mask, position_ids, Wq, Wk, Wv, Wo,
           _trace=False, _tmpdir=None):
    from concourse.bass_utils import run_bass_kernel_spmd

    hidden = np.ascontiguousarray(
        np.asarray(hidden_states, dtype=np.float32).reshape(T, HID))
    Wq = np.asarray(Wq, dtype=np.float32)
    Wk = np.asarray(Wk, dtype=np.float32)
    Wv = np.asarray(Wv, dtype=np.float32)
    Wo = np.asarray(Wo, dtype=np.float32)
    cos_t, sin_t = _host_tables(position_ids)
    msk = _host_masks(attention_mask)
    ident = np.eye(128, dtype=np.float32)

    nc = _get_nc()
    in_maps = []
    for c in range(NCORES):
        in_maps.append({
            "hidden": hidden,
            "wq": np.ascontiguousarray(Wq[:, c * QH * HD:(c + 1) * QH * HD]),
            "wk": np.ascontiguousarray(Wk[:, c * HD:(c + 1) * HD]),
            "wv": np.ascontiguousarray(Wv[:, c * VD:(c + 1) * VD]),
            "wo": np.ascontiguousarray(Wo[c * OSH:(c + 1) * OSH, :]),
            "cos": cos_t, "sin": sin_t,
            "masks": msk, "ident": ident,
        })
    res = run_bass_kernel_spmd(nc, in_maps, list(range(NCORES)),
                               trace=_trace, tmpdir=_tmpdir)
    parts = [res.results[c]["out"] for c in range(NCORES)]
    full = parts[0].copy()
    for p in parts[1:]:
        full += p                      # [CO, T] partial sums
    out = np.ascontiguousarray(full.T).reshape(B, S, CO)
    if _trace:
        kernel.last_exec_time_ns = res.exec_time_ns
        kernel.last_profile = res.profile_json
    return out



# revision 9
# speedup vs baseline: 1.3664x; 1.3664x over previous
"""MiMoV2 GQA attention (B=2, S=2048, HID=4096, 32 Q heads / 8 KV heads,
HD=128, VD=96, partial RoPE 64, causal) on 8 TRN2 NeuronCores.

Sharding: tensor-parallel over heads. Core c owns Q heads 4c..4c+3 and KV
head c (Wq/Wk/Wv column shards, Wo row shard). Activations replicated; the
row-parallel Wo partial outputs are summed on the host (the unshard step).

v2 vs baseline:
  * hidden is pre-transposed AND pre-cast to bf16 on the host, so phase A
    streams hiddenT chunks straight into the QKV matmuls -- no PE
    transposes, no DVE casts of transposed tiles.
  * every matmul operand is bf16 (1 cy/row like fp32r at N>=256, but half
    the DMA traffic and 1.0 cy/row transposes); PSUM accumulates in f32.
  * Q/K/V stages and the attention working set live in SBUF (no DRAM
    round-trip); only the attention output (at) is staged through DRAM for
    the partition-realignment that phase C needs.
  * phase B fuses exp over [128,1024] two-bank PSUM score tiles to amortize
    the ~240ns/op ACT overhead; the diagonal 128-col slivers are masked
    in-place on DVE with a single shared [128,128] triangle mask.
  * the softmax denominator rides as a ones-column in V (acc row 96); all
    four 512-wide denominator rows of a (b,qh) are packed into a [4,512]
    tile for one reciprocal, then broadcast per 512-block with a K=1
    matmul against a [4,96] ones tile (base partitions line up).
  * phase C runs ct-outer / oc-mid / tb-inner so each Wo stationary tile is
    reused for 8 consecutive matmuls, with drain-as-you-go on all 8 PSUM
    banks; output tiles are written bf16 (host sums partials in f32).
"""
import numpy as np

import concourse.bacc as bacc
import concourse.mybir as mybir
import concourse.tile as tile

F32 = mybir.dt.float32
F32R = mybir.dt.float32r
BF16 = mybir.dt.bfloat16

B, S, HID = 2, 2048, 4096
NH, NKV, HD, VD = 32, 8, 128, 96
ROPE = 64
NCORES = 8
QH = NH // NCORES            # 4 q heads per core
T = B * S                    # 4096 tokens
OSH = QH * VD                # 384 output dims per core
CO = NH * VD                 # 3072 full output dim
THETA = 1000000.0
SCALE = float(HD ** -0.5)
NVB = S // 128               # 16 v blocks per batch

AFT = mybir.ActivationFunctionType


def _build():
    nc = bacc.Bacc("TRN2", target_bir_lowering=False, debug=False,
                   num_devices=NCORES)
    hiddent = nc.declare_dram_parameter("hiddent", [HID, T], BF16, False)
    wq = nc.declare_dram_parameter("wq", [HID, QH * HD], BF16, False)
    wk = nc.declare_dram_parameter("wk", [HID, HD], BF16, False)
    wv = nc.declare_dram_parameter("wv", [HID, VD], BF16, False)
    wo = nc.declare_dram_parameter("wo", [OSH, CO], BF16, False)
    cos = nc.declare_dram_parameter("cos", [ROPE, T], F32, False)
    sin = nc.declare_dram_parameter("sin", [ROPE, T], F32, False)
    tri = nc.declare_dram_parameter("tri", [128, 128], BF16, False)
    ident = nc.declare_dram_parameter("ident", [128, 128], BF16, False)
    out = nc.declare_dram_parameter("out", [CO, T], BF16, True)

    at_stage = nc.dram_tensor("at_stage", [OSH, T], BF16)

    hT_r = hiddent.rearrange("(c p) t -> p c t", p=128)
    wq_r = wq.rearrange("(c p) o -> p c o", p=128)
    wk_r = wk.rearrange("(c p) o -> p c o", p=128)
    wv_r = wv.rearrange("(c p) o -> p c o", p=128)
    wo_r = wo.rearrange("(c p) o -> p c o", p=128)
    at_r = at_stage.rearrange("(c p) t -> p c t", p=128)

    with tile.TileContext(nc) as tc:
        with (
            tc.tile_pool(name="cst", bufs=1) as cst,
            tc.tile_pool(name="stg", bufs=1) as stg,
        ):
            id_sb = cst.tile([128, 128], BF16, tag="ident")
            nc.sync.dma_start(out=id_sb[:], in_=ident[:])
            msk_sb = cst.tile([128, 128], BF16, tag="msk")
            nc.sync.dma_start(out=msk_sb[:], in_=tri[:])
            ones96_f = cst.tile([1, 96], F32, tag="ones96_f")
            nc.gpsimd.memset(ones96_f[:], 1.0)
            ones96 = cst.tile([1, 96], F32R, tag="ones96")
            nc.vector.tensor_copy(ones96[:], ones96_f[:])

            # persistent stages (SBUF-resident across phases)
            wo_sb = stg.tile([128, OSH // 128, CO], BF16, tag="wo")
            nc.sync.dma_start(out=wo_sb[:], in_=wo_r[:])
            at_sb = stg.tile([128, OSH // 128, T], BF16, tag="atall")
            qs = [[stg.tile([128, S], BF16, tag=f"qs{b}_{h}",
                            name=f"qs{b}_{h}") for h in range(QH)]
                  for b in range(B)]
            ks = [stg.tile([128, S], BF16, tag=f"ks{b}", name=f"ks{b}")
                  for b in range(B)]
            vx = [stg.tile([128, NVB * (VD + 1)], BF16, tag=f"vx{b}",
                           name=f"vx{b}") for b in range(B)]
            for b in range(B):
                nc.gpsimd.memset(vx[b][:], 1.0)   # ones columns survive

            # ---------------- Phase A: QKV projections + RoPE ------------
            with (
                tc.tile_pool(name="wpool", bufs=1) as wpool,
                tc.tile_pool(name="hpool", bufs=3) as hpool,
                tc.tile_pool(name="rpool", bufs=3) as rpool,
                tc.tile_pool(name="apsum", bufs=1, space="PSUM") as apsum,
                tc.tile_pool(name="vtps", bufs=2, space="PSUM") as vtps,
            ):
                cos_sb = wpool.tile([ROPE, T], F32, tag="cos")
                nc.sync.dma_start(out=cos_sb[:], in_=cos[:])
                sin_sb = wpool.tile([ROPE, T], F32, tag="sin")
                nc.sync.dma_start(out=sin_sb[:], in_=sin[:])
                wq_sb = wpool.tile([128, HID // 128, QH * HD], BF16,
                                   tag="wq")
                for g in range(8):
                    gsl = slice(g * 4, (g + 1) * 4)
                    nc.sync.dma_start(out=wq_sb[:, gsl, :],
                                      in_=wq_r[:, gsl, :])
                wk_sb = wpool.tile([128, HID // 128, HD], BF16, tag="wk")
                nc.sync.dma_start(out=wk_sb[:], in_=wk_r[:])
                wv_sb = wpool.tile([128, HID // 128, VD], BF16, tag="wv")
                nc.sync.dma_start(out=wv_sb[:], in_=wv_r[:])

                def rope_finish(src, dst, stsl, tsl, nm):
                    # dst[0:64] = src[0:64]*cos + rot(src)*sin ; rest cast
                    t1 = rpool.tile([ROPE, 512], F32, tag="t1",
                                    name=f"t1_{nm}")
                    nc.vector.tensor_mul(t1[0:32, :], src[32:64, :],
                                         sin_sb[32:64, tsl])
                    nc.vector.tensor_mul(t1[32:64, :], src[0:32, :],
                                         sin_sb[0:32, tsl])
                    qc = rpool.tile([ROPE, 512], F32, tag="qc",
                                    name=f"qc_{nm}")
                    nc.vector.tensor_mul(qc[:], src[0:ROPE, :],
                                         cos_sb[:, tsl])
                    nc.vector.tensor_add(dst[0:ROPE, stsl], qc[:], t1[:])
                    nc.scalar.copy(dst[ROPE:128, stsl], src[ROPE:128, :])

                for tt in range(T // 512):
                    b_, st4 = tt // 4, tt % 4
                    stsl = slice(st4 * 512, st4 * 512 + 512)
                    tsl = slice(tt * 512, (tt + 1) * 512)
                    qacc = [apsum.tile([128, 512], F32, tag=f"qacc{h}",
                                       name=f"qacc{h}_{tt}")
                            for h in range(QH)]
                    kacc = apsum.tile([128, 512], F32, tag="kacc",
                                      name=f"kacc_{tt}")
                    vacc = apsum.tile([VD, 512], F32, tag="vacc",
                                      name=f"vacc_{tt}")
                    for g in range(8):
                        ld = hpool.tile([128, 4, 512], BF16, tag="h",
                                        name=f"ld_{tt}_{g}")
                        eng = nc.sync if g % 2 == 0 else nc.gpsimd
                        eng.dma_start(out=ld[:],
                                      in_=hT_r[:, g * 4:(g + 1) * 4, tsl])
                        for c4 in range(4):
                            hc = g * 4 + c4
                            st_, sp_ = hc == 0, hc == HID // 128 - 1
                            rhs = ld[:, c4, :]
                            nc.tensor.matmul(kacc[:], wk_sb[:, hc, :], rhs,
                                             start=st_, stop=sp_)
                            nc.tensor.matmul(vacc[:], wv_sb[:, hc, :], rhs,
                                             start=st_, stop=sp_)
                            for h in range(QH):
                                nc.tensor.matmul(
                                    qacc[h][:],
                                    wq_sb[:, hc, h * 128:(h + 1) * 128],
                                    rhs, start=st_, stop=sp_)
                    # boundary: K first (phase B consumes it first), then V
                    rope_finish(kacc, ks[b_], stsl, tsl, f"k{tt}")
                    vts = rpool.tile([VD, 512], BF16, tag="vts",
                                     name=f"vts_{tt}")
                    nc.scalar.copy(vts[:], vacc[:])
                    for h in range(QH):
                        rope_finish(qacc[h], qs[b_][h], stsl, tsl,
                                    f"q{tt}_{h}")
                    for sub in range(4):
                        vtp = vtps.tile([128, VD], BF16, tag="vtp",
                                        name=f"vtp_{tt}_{sub}")
                        nc.tensor.matmul(vtp[:],
                                         vts[:, sub * 128:(sub + 1) * 128],
                                         id_sb[0:VD, 0:VD], start=True,
                                         stop=True, is_transpose=True)
                        blk = st4 * 4 + sub
                        nc.vector.tensor_copy(
                            vx[b_][:, blk * (VD + 1):blk * (VD + 1) + VD],
                            vtp[:])

            # ---------------- Phase B: causal attention ------------------
            with (
                tc.tile_pool(name="bpool", bufs=2) as bpool,
                tc.tile_pool(name="ptpool", bufs=3) as ptpool,
                tc.tile_pool(name="stps", bufs=2, space="PSUM") as stps,
                tc.tile_pool(name="accb", bufs=2, space="PSUM") as accb,
                tc.tile_pool(name="bcps", bufs=2, space="PSUM") as bcps,
            ):
                def emit_pv(p):
                    acc_, pt2_, jp_, ib_, last_ = p
                    first = jp_ == 0
                    if jp_ >= 2 * ib_:          # diagonal pair
                        s0 = 2 * (jp_ - 2 * ib_)
                        r0, r1 = s0 * 128, s0 * 128 + 128
                        j0 = 4 * ib_ + s0
                        nc.tensor.matmul(
                            acc_[:, r0:512],
                            vxb[:, j0 * 97:j0 * 97 + 97],
                            pt2_[:, r0:512], start=first, stop=False)
                        nc.tensor.matmul(
                            acc_[:, r1:512],
                            vxb[:, (j0 + 1) * 97:(j0 + 1) * 97 + 97],
                            pt2_[:, 512 + r1:1024], start=False, stop=last_)
                    else:
                        j0 = 2 * jp_
                        nc.tensor.matmul(acc_[:],
                                         vxb[:, j0 * 97:j0 * 97 + 97],
                                         pt2_[:, 0:512], start=first,
                                         stop=False)
                        nc.tensor.matmul(
                            acc_[:],
                            vxb[:, (j0 + 1) * 97:(j0 + 1) * 97 + 97],
                            pt2_[:, 512:1024], start=False, stop=last_)

                for b in range(B):
                    ksb, vxb = ks[b], vx[b]
                    for qh in range(QH):
                        dn4 = bpool.tile([97, 512], F32, tag="dn4",
                                         name=f"dn4_{b}_{qh}")
                        atus = []
                        pend = None
                        prev_acc = None

                        def drain_prev(ib_):
                            nc.vector.tensor_copy(
                                dn4[32 * ib_:32 * ib_ + 1, :],
                                prev_acc[VD:VD + 1, :])
                            atu = bpool.tile([VD, 512], BF16, tag="atu",
                                             bufs=8,
                                             name=f"atu_{b}_{qh}_{ib_}")
                            nc.vector.tensor_copy(atu[:], prev_acc[0:VD, :])
                            atus.append(atu)

                        for ib in range(4):
                            qcols = qs[b][qh][:, ib * 512:(ib + 1) * 512]
                            acc = accb.tile([VD + 1, 512], F32, tag="acc",
                                            name=f"acc_{b}_{qh}_{ib}")
                            npair = 2 * ib + 2
                            for jp in range(npair):
                                stp2 = stps.tile([128, 1024], F32,
                                                 tag="stp")
                                pt2 = ptpool.tile([128, 1024], BF16,
                                                  tag="pt")
                                if jp >= 2 * ib:     # diagonal pair
                                    s0 = 2 * (jp - 2 * ib)
                                    r0, r1 = s0 * 128, s0 * 128 + 128
                                    j0 = 4 * ib + s0
                                    nc.tensor.matmul(
                                        stp2[:, r0:512],
                                        ksb[:, j0 * 128:(j0 + 1) * 128],
                                        qcols[:, r0:512], start=True,
                                        stop=True)
                                    nc.tensor.matmul(
                                        stp2[:, 512 + r1:1024],
                                        ksb[:, (j0 + 1) * 128:
                                            (j0 + 2) * 128],
                                        qcols[:, r1:512], start=True,
                                        stop=True)
                                    nc.scalar.activation(
                                        pt2[:, r0:512], stp2[:, r0:512],
                                        AFT.Exp, scale=SCALE)
                                    nc.scalar.activation(
                                        pt2[:, 512 + r1:1024],
                                        stp2[:, 512 + r1:1024],
                                        AFT.Exp, scale=SCALE)
                                    nc.vector.tensor_mul(
                                        pt2[:, r0:r0 + 128],
                                        pt2[:, r0:r0 + 128], msk_sb[:])
                                    nc.vector.tensor_mul(
                                        pt2[:, 512 + r1:512 + r1 + 128],
                                        pt2[:, 512 + r1:512 + r1 + 128],
                                        msk_sb[:])
                                else:                # two full blocks
                                    j0 = 2 * jp
                                    nc.tensor.matmul(
                                        stp2[:, 0:512],
                                        ksb[:, j0 * 128:(j0 + 1) * 128],
                                        qcols, start=True, stop=True)
                                    nc.tensor.matmul(
                                        stp2[:, 512:1024],
                                        ksb[:, (j0 + 1) * 128:
                                            (j0 + 2) * 128],
                                        qcols, start=True, stop=True)
                                    nc.scalar.activation(
                                        pt2[:], stp2[:], AFT.Exp,
                                        scale=SCALE)
                                if pend is not None:
                                    emit_pv(pend)
                                if jp == 1 and prev_acc is not None:
                                    # previous ib's acc closed at jp==0's
                                    # flush; drain it on DVE under this
                                    # ib's matmuls
                                    drain_prev(ib - 1)
                                    prev_acc = None
                                pend = (acc, pt2, jp, ib, jp == npair - 1)
                            prev_acc = acc
                        emit_pv(pend)
                        drain_prev(3)
                        rec4 = bpool.tile([97, 512], F32, tag="rec4",
                                          name=f"rec4_{b}_{qh}")
                        nc.vector.reciprocal(rec4[:], dn4[:])
                        for ib in range(4):
                            rr = bpool.tile([1, 512], F32R, tag="rr",
                                            bufs=4,
                                            name=f"rr_{b}_{qh}_{ib}")
                            nc.vector.tensor_copy(
                                rr[:], rec4[32 * ib:32 * ib + 1, :])
                            bcp = bcps.tile([VD, 512], F32, tag="bcp",
                                            name=f"bcp_{b}_{qh}_{ib}")
                            nc.tensor.matmul(bcp[:], ones96[:],
                                             rr[:],
                                             start=True, stop=True)
                            at_t = bpool.tile([VD, 512], BF16, tag="att",
                                              bufs=4,
                                              name=f"att_{b}_{qh}_{ib}")
                            nc.vector.tensor_mul(at_t[:], atus[ib][:],
                                                 bcp[:])
                            nc.gpsimd.dma_start(
                                out=at_stage[qh * VD:(qh + 1) * VD,
                                             b * S + ib * 512:
                                             b * S + (ib + 1) * 512],
                                in_=at_t[:])
                    # prefetch this batch's at columns for phase C
                    nc.sync.dma_start(out=at_sb[:, :, b * S:(b + 1) * S],
                                      in_=at_r[:, :, b * S:(b + 1) * S])

            # ---------------- Phase C: output projection -----------------
            with (
                tc.tile_pool(name="opool", bufs=6) as opool,
                tc.tile_pool(name="cps", bufs=1, space="PSUM") as cps,
            ):
                for ct in range(CO // 128):
                    ops = [cps.tile([128, 512], F32, tag=f"c{tb}",
                                    name=f"ops{ct}_{tb}")
                           for tb in range(T // 512)]
                    for oc in range(OSH // 128):
                        for tb in range(T // 512):
                            nc.tensor.matmul(
                                ops[tb][:],
                                wo_sb[:, oc, ct * 128:(ct + 1) * 128],
                                at_sb[:, oc, tb * 512:(tb + 1) * 512],
                                start=(oc == 0), stop=(oc == OSH // 128 - 1))
                    for tb in range(T // 512):
                        osb = opool.tile([128, 512], BF16, tag="osb",
                                         name=f"osb_{ct}_{tb}")
                        if tb % 3 == 1:
                            nc.scalar.copy(osb[:], ops[tb][:])
                        else:
                            nc.vector.tensor_copy(osb[:], ops[tb][:])
                        nc.gpsimd.dma_start(
                            out=out[ct * 128:(ct + 1) * 128,
                                    tb * 512:(tb + 1) * 512],
                            in_=osb[:])

    nc.compile()
    return nc


_NC_CACHE = None


def _get_nc():
    global _NC_CACHE
    if _NC_CACHE is None:
        _NC_CACHE = _build()
    return _NC_CACHE


def _host_tables(position_ids):
    pos = np.asarray(position_ids).reshape(-1)[:S].astype(np.float64)
    inv_freq = 1.0 / (THETA ** (np.arange(0, ROPE, 2, dtype=np.float64) / ROPE))
    freqs = np.outer(pos, inv_freq)                       # [S, ROPE/2]
    emb = np.concatenate([freqs, freqs], axis=-1)         # [S, ROPE]
    cos_t = np.tile(np.cos(emb).astype(np.float32).T, (1, B))  # [ROPE, T]
    sinf = np.sin(freqs).astype(np.float32).T                  # [ROPE/2, S]
    # rows 0:32 = +sin (t1[32:64] = q[0:32]*sin)
    # rows 32:64 = -sin (t1[0:32] = -q[32:64]*sin)
    sin_t = np.tile(np.concatenate([sinf, -sinf], axis=0), (1, B))
    return cos_t, sin_t


def kernel(hidden_states, attention_mask, position_ids, Wq, Wk, Wv, Wo,
           _trace=False, _tmpdir=None):
    import ml_dtypes
    from concourse.bass_utils import run_bass_kernel_spmd
    bf16 = ml_dtypes.bfloat16

    hidden = np.asarray(hidden_states, dtype=np.float32).reshape(T, HID)
    hiddent = np.ascontiguousarray(hidden.T).astype(bf16)
    Wq = np.asarray(Wq, dtype=np.float32)
    Wk = np.asarray(Wk, dtype=np.float32)
    Wv = np.asarray(Wv, dtype=np.float32)
    Wo = np.asarray(Wo, dtype=np.float32)
    cos_t, sin_t = _host_tables(position_ids)
    m = np.asarray(attention_mask).reshape(S, S)
    tri = np.ascontiguousarray((m[0:128, 0:128] == 0.0).T).astype(bf16)
    ident = np.eye(128, dtype=np.float32).astype(bf16)

    nc = _get_nc()
    in_maps = []
    for c in range(NCORES):
        in_maps.append({
            "hiddent": hiddent,
            "wq": np.ascontiguousarray(
                Wq[:, c * QH * HD:(c + 1) * QH * HD]).astype(bf16),
            "wk": np.ascontiguousarray(Wk[:, c * HD:(c + 1) * HD]).astype(bf16),
            "wv": np.ascontiguousarray(Wv[:, c * VD:(c + 1) * VD]).astype(bf16),
            "wo": np.ascontiguousarray(Wo[c * OSH:(c + 1) * OSH, :]).astype(bf16),
            "cos": cos_t, "sin": sin_t,
            "tri": tri, "ident": ident,
        })
    res = run_bass_kernel_spmd(nc, in_maps, list(range(NCORES)),
                               trace=_trace, tmpdir=_tmpdir)
    full = np.zeros((CO, T), dtype=np.float32)
    for c in range(NCORES):
        full += res.results[c]["out"].astype(np.float32)
    out = np.ascontiguousarray(full.T).reshape(B, S, CO)
    if _trace:
        kernel.last_exec_time_ns = res.exec_time_ns
        kernel.last_profile = res.profile_json
    return out


# revision 16
# speedup vs baseline: 1.4617x; 1.0697x over previous
"""MiMoV2 GQA attention (B=2, S=2048, HID=4096, 32 Q heads / 8 KV heads,
HD=128, VD=96, partial RoPE 64, causal) on 8 TRN2 NeuronCores.

Sharding: tensor-parallel over heads. Core c owns Q heads 4c..4c+3 and KV
head c (Wq/Wk/Wv column shards, Wo row shard). Activations replicated; the
row-parallel Wo partial outputs are summed on the host (the unshard step).

v2 vs baseline:
  * hidden is pre-transposed AND pre-cast to bf16 on the host, so phase A
    streams hiddenT chunks straight into the QKV matmuls -- no PE
    transposes, no DVE casts of transposed tiles.
  * every matmul operand is bf16 (1 cy/row like fp32r at N>=256, but half
    the DMA traffic and 1.0 cy/row transposes); PSUM accumulates in f32.
  * Q/K/V stages and the attention working set live in SBUF (no DRAM
    round-trip); only the attention output (at) is staged through DRAM for
    the partition-realignment that phase C needs.
  * phase B fuses exp over [128,1024] two-bank PSUM score tiles to amortize
    the ~240ns/op ACT overhead; the diagonal 128-col slivers are masked
    in-place on DVE with a single shared [128,128] triangle mask.
  * the softmax denominator rides as a ones-column in V (acc row 96); all
    four 512-wide denominator rows of a (b,qh) are packed into a [4,512]
    tile for one reciprocal, then broadcast per 512-block with a K=1
    matmul against a [4,96] ones tile (base partitions line up).
  * phase C runs ct-outer / oc-mid / tb-inner so each Wo stationary tile is
    reused for 8 consecutive matmuls, with drain-as-you-go on all 8 PSUM
    banks; output tiles are written bf16 (host sums partials in f32).
"""
import numpy as np

import concourse.bacc as bacc
import concourse.mybir as mybir
import concourse.tile as tile

F32 = mybir.dt.float32
F32R = mybir.dt.float32r
BF16 = mybir.dt.bfloat16

B, S, HID = 2, 2048, 4096
NH, NKV, HD, VD = 32, 8, 128, 96
ROPE = 64
NCORES = 8
QH = NH // NCORES            # 4 q heads per core
T = B * S                    # 4096 tokens
OSH = QH * VD                # 384 output dims per core
CO = NH * VD                 # 3072 full output dim
THETA = 1000000.0
SCALE = float(HD ** -0.5)
NVB = S // 128               # 16 v blocks per batch

AFT = mybir.ActivationFunctionType


def _build():
    nc = bacc.Bacc("TRN2", target_bir_lowering=False, debug=False,
                   num_devices=NCORES)
    hiddent = nc.declare_dram_parameter("hiddent", [HID, T], BF16, False)
    wq = nc.declare_dram_parameter("wq", [HID, QH * HD], BF16, False)
    wk = nc.declare_dram_parameter("wk", [HID, HD], BF16, False)
    wv = nc.declare_dram_parameter("wv", [HID, VD], BF16, False)
    wo = nc.declare_dram_parameter("wo", [OSH, CO], BF16, False)
    cos = nc.declare_dram_parameter("cos", [ROPE, T], F32, False)
    sin = nc.declare_dram_parameter("sin", [ROPE, T], F32, False)
    tri = nc.declare_dram_parameter("tri", [128, 128], BF16, False)
    ident = nc.declare_dram_parameter("ident", [128, 128], BF16, False)
    out = nc.declare_dram_parameter("out", [CO, T], BF16, True)

    hT_r = hiddent.rearrange("(c p) t -> p c t", p=128)
    wq_r = wq.rearrange("(c p) o -> p c o", p=128)
    wk_r = wk.rearrange("(c p) o -> p c o", p=128)
    wv_r = wv.rearrange("(c p) o -> p c o", p=128)
    wo_r = wo.rearrange("(c p) o -> p c o", p=128)

    # at output rows qh*96..qh*96+96 split into [128,·] chunk segments:
    # (chunk, part_lo, part_hi, vd_lo, vd_hi) per qh.  Every source and
    # destination partition window must start at a multiple of 32 and not
    # cross its natural alignment block (64@32 is illegal, 32@32 is fine).
    AT_SEGS = {
        0: [(0, 0, 96, 0, 96)],
        1: [(0, 96, 128, 0, 32), (1, 0, 32, 32, 64), (1, 32, 64, 64, 96)],
        2: [(1, 64, 128, 0, 64), (2, 0, 32, 64, 96)],
        3: [(2, 32, 64, 0, 32), (2, 64, 96, 32, 64), (2, 96, 128, 64, 96)],
    }

    with tile.TileContext(nc) as tc:
        with (
            tc.tile_pool(name="cst", bufs=1) as cst,
            tc.tile_pool(name="stg", bufs=1) as stg,
        ):
            id_sb = cst.tile([128, 128], BF16, tag="ident")
            nc.sync.dma_start(out=id_sb[:], in_=ident[:])
            msk_sb = cst.tile([128, 128], BF16, tag="msk")
            nc.sync.dma_start(out=msk_sb[:], in_=tri[:])
            ones96_f = cst.tile([1, 96], F32, tag="ones96_f")
            nc.gpsimd.memset(ones96_f[:], 1.0)
            ones96 = cst.tile([1, 96], F32R, tag="ones96")
            nc.vector.tensor_copy(ones96[:], ones96_f[:])

            # persistent stages (SBUF-resident across phases)
            wo_sb = stg.tile([128, OSH // 128, CO], BF16, tag="wo")
            nc.sync.dma_start(out=wo_sb[:], in_=wo_r[:])
            at_sb = stg.tile([128, OSH // 128, T], BF16, tag="atall")
            qs = [[stg.tile([128, S], BF16, tag=f"qs{b}_{h}",
                            name=f"qs{b}_{h}") for h in range(QH)]
                  for b in range(B)]
            ks = [stg.tile([128, S], BF16, tag=f"ks{b}", name=f"ks{b}")
                  for b in range(B)]
            vx = [stg.tile([128, NVB * (VD + 1)], BF16, tag=f"vx{b}",
                           name=f"vx{b}") for b in range(B)]
            for b in range(B):
                nc.gpsimd.memset(vx[b][:], 1.0)   # ones columns survive

            # ---------------- Phase A: QKV projections + RoPE ------------
            with (
                tc.tile_pool(name="wpool", bufs=1) as wpool,
                tc.tile_pool(name="hpool", bufs=5) as hpool,
                tc.tile_pool(name="rpool", bufs=3) as rpool,
                tc.tile_pool(name="apsum", bufs=1, space="PSUM") as apsum,
                tc.tile_pool(name="vtps", bufs=2, space="PSUM") as vtps,
            ):
                cos_sb = wpool.tile([ROPE, T], F32, tag="cos")
                nc.sync.dma_start(out=cos_sb[:], in_=cos[:])
                sin_sb = wpool.tile([ROPE, T], F32, tag="sin")
                nc.sync.dma_start(out=sin_sb[:], in_=sin[:])
                wq_sb = wpool.tile([128, HID // 128, QH * HD], BF16,
                                   tag="wq")
                for g in range(8):
                    gsl = slice(g * 4, (g + 1) * 4)
                    nc.sync.dma_start(out=wq_sb[:, gsl, :],
                                      in_=wq_r[:, gsl, :])
                wk_sb = wpool.tile([128, HID // 128, HD], BF16, tag="wk")
                nc.sync.dma_start(out=wk_sb[:], in_=wk_r[:])
                wv_sb = wpool.tile([128, HID // 128, VD], BF16, tag="wv")
                nc.sync.dma_start(out=wv_sb[:], in_=wv_r[:])

                def rope_finish(src, dst, stsl, tsl, nm):
                    # dst[0:64] = src[0:64]*cos + rot(src)*sin ; rest cast
                    t1 = rpool.tile([ROPE, 512], F32, tag="t1",
                                    name=f"t1_{nm}")
                    nc.vector.tensor_mul(t1[0:32, :], src[32:64, :],
                                         sin_sb[32:64, tsl])
                    nc.vector.tensor_mul(t1[32:64, :], src[0:32, :],
                                         sin_sb[0:32, tsl])
                    qc = rpool.tile([ROPE, 512], F32, tag="qc",
                                    name=f"qc_{nm}")
                    nc.vector.tensor_mul(qc[:], src[0:ROPE, :],
                                         cos_sb[:, tsl])
                    nc.vector.tensor_add(dst[0:ROPE, stsl], qc[:], t1[:])
                    nc.scalar.copy(dst[ROPE:128, stsl], src[ROPE:128, :])

                for tt in range(T // 512):
                    b_, st4 = tt // 4, tt % 4
                    stsl = slice(st4 * 512, st4 * 512 + 512)
                    tsl = slice(tt * 512, (tt + 1) * 512)
                    qacc = [apsum.tile([128, 512], F32, tag=f"qacc{h}",
                                       name=f"qacc{h}_{tt}")
                            for h in range(QH)]
                    kacc = apsum.tile([128, 512], F32, tag="kacc",
                                      name=f"kacc_{tt}")
                    vacc = apsum.tile([VD, 512], F32, tag="vacc",
                                      name=f"vacc_{tt}")
                    for g in range(8):
                        ld = hpool.tile([128, 4, 512], BF16, tag="h",
                                        name=f"ld_{tt}_{g}")
                        eng = nc.sync if g % 2 == 0 else nc.gpsimd
                        eng.dma_start(out=ld[:],
                                      in_=hT_r[:, g * 4:(g + 1) * 4, tsl])
                        for c4 in range(4):
                            hc = g * 4 + c4
                            st_, sp_ = hc == 0, hc == HID // 128 - 1
                            rhs = ld[:, c4, :]
                            nc.tensor.matmul(kacc[:], wk_sb[:, hc, :], rhs,
                                             start=st_, stop=sp_)
                            nc.tensor.matmul(vacc[:], wv_sb[:, hc, :], rhs,
                                             start=st_, stop=sp_)
                            for h in range(QH):
                                nc.tensor.matmul(
                                    qacc[h][:],
                                    wq_sb[:, hc, h * 128:(h + 1) * 128],
                                    rhs, start=st_, stop=sp_)
                    # boundary: K first (phase B consumes it first), then V
                    rope_finish(kacc, ks[b_], stsl, tsl, f"k{tt}")
                    vts = rpool.tile([VD, 512], BF16, tag="vts",
                                     name=f"vts_{tt}")
                    nc.scalar.copy(vts[:], vacc[:])
                    for h in range(QH):
                        rope_finish(qacc[h], qs[b_][h], stsl, tsl,
                                    f"q{tt}_{h}")
                    for sub in range(4):
                        vtp = vtps.tile([128, VD], BF16, tag="vtp",
                                        name=f"vtp_{tt}_{sub}")
                        nc.tensor.matmul(vtp[:],
                                         vts[:, sub * 128:(sub + 1) * 128],
                                         id_sb[0:VD, 0:VD], start=True,
                                         stop=True, is_transpose=True)
                        blk = st4 * 4 + sub
                        nc.vector.tensor_copy(
                            vx[b_][:, blk * (VD + 1):blk * (VD + 1) + VD],
                            vtp[:])

            # ---------------- Phase B: causal attention ------------------
            with (
                tc.tile_pool(name="bpool", bufs=2) as bpool,
                tc.tile_pool(name="ptpool", bufs=3) as ptpool,
                tc.tile_pool(name="stps", bufs=2, space="PSUM") as stps,
                tc.tile_pool(name="accb", bufs=2, space="PSUM") as accb,
                tc.tile_pool(name="bcps", bufs=2, space="PSUM") as bcps,
            ):
                def emit_pv(p):
                    acc_, pt2_, jp_, ib_, last_ = p
                    first = jp_ == 0
                    if jp_ >= 2 * ib_:          # diagonal pair
                        s0 = 2 * (jp_ - 2 * ib_)
                        r0, r1 = s0 * 128, s0 * 128 + 128
                        j0 = 4 * ib_ + s0
                        nc.tensor.matmul(
                            acc_[:, r0:512],
                            vxb[:, j0 * 97:j0 * 97 + 97],
                            pt2_[:, r0:512], start=first, stop=False)
                        nc.tensor.matmul(
                            acc_[:, r1:512],
                            vxb[:, (j0 + 1) * 97:(j0 + 1) * 97 + 97],
                            pt2_[:, 512 + r1:1024], start=False, stop=last_)
                    else:
                        j0 = 2 * jp_
                        nc.tensor.matmul(acc_[:],
                                         vxb[:, j0 * 97:j0 * 97 + 97],
                                         pt2_[:, 0:512], start=first,
                                         stop=False)
                        nc.tensor.matmul(
                            acc_[:],
                            vxb[:, (j0 + 1) * 97:(j0 + 1) * 97 + 97],
                            pt2_[:, 512:1024], start=False, stop=last_)

                norm_tail = [None]

                def flush_norm():
                    if norm_tail[0] is None:
                        return
                    b_, qh_, dn4_, atus_ = norm_tail[0]
                    norm_tail[0] = None
                    rec4 = bpool.tile([97, 512], F32, tag="rec4",
                                      name=f"rec4_{b_}_{qh_}")
                    nc.vector.reciprocal(rec4[:], dn4_[:])
                    for ib_ in range(4):
                        rr = bpool.tile([1, 512], F32R, tag="rr",
                                        bufs=4, name=f"rr_{b_}_{qh_}_{ib_}")
                        nc.vector.tensor_copy(
                            rr[:], rec4[32 * ib_:32 * ib_ + 1, :])
                        bcp = bcps.tile([VD, 512], F32, tag="bcp",
                                        name=f"bcp_{b_}_{qh_}_{ib_}")
                        nc.tensor.matmul(bcp[:], ones96[:], rr[:],
                                         start=True, stop=True)
                        csl = slice(b_ * S + ib_ * 512,
                                    b_ * S + (ib_ + 1) * 512)
                        for (c, pa, pb, va, vb_) in AT_SEGS[qh_]:
                            nc.vector.tensor_mul(at_sb[pa:pb, c, csl],
                                                 atus_[ib_][va:vb_, :],
                                                 bcp[va:vb_, :])

                for b in range(B):
                    ksb, vxb = ks[b], vx[b]
                    for qh in range(QH):
                        dn4 = bpool.tile([97, 512], F32, tag="dn4",
                                         name=f"dn4_{b}_{qh}")
                        atus = []
                        pend = None
                        prev_acc = None

                        def drain_prev(ib_):
                            nc.vector.tensor_copy(
                                dn4[32 * ib_:32 * ib_ + 1, :],
                                prev_acc[VD:VD + 1, :])
                            atu = bpool.tile([VD, 512], BF16, tag="atu",
                                             bufs=8,
                                             name=f"atu_{b}_{qh}_{ib_}")
                            nc.vector.tensor_copy(atu[:], prev_acc[0:VD, :])
                            atus.append(atu)

                        for ib in range(4):
                            qcols = qs[b][qh][:, ib * 512:(ib + 1) * 512]
                            acc = accb.tile([VD + 1, 512], F32, tag="acc",
                                            name=f"acc_{b}_{qh}_{ib}")
                            npair = 2 * ib + 2
                            for jp in range(npair):
                                stp2 = stps.tile([128, 1024], F32,
                                                 tag="stp")
                                pt2 = ptpool.tile([128, 1024], BF16,
                                                  tag="pt")
                                if jp >= 2 * ib:     # diagonal pair
                                    s0 = 2 * (jp - 2 * ib)
                                    r0, r1 = s0 * 128, s0 * 128 + 128
                                    j0 = 4 * ib + s0
                                    nc.tensor.matmul(
                                        stp2[:, r0:512],
                                        ksb[:, j0 * 128:(j0 + 1) * 128],
                                        qcols[:, r0:512], start=True,
                                        stop=True)
                                    nc.tensor.matmul(
                                        stp2[:, 512 + r1:1024],
                                        ksb[:, (j0 + 1) * 128:
                                            (j0 + 2) * 128],
                                        qcols[:, r1:512], start=True,
                                        stop=True)
                                    nc.scalar.activation(
                                        pt2[:, r0:512], stp2[:, r0:512],
                                        AFT.Exp, scale=SCALE)
                                    nc.scalar.activation(
                                        pt2[:, 512 + r1:1024],
                                        stp2[:, 512 + r1:1024],
                                        AFT.Exp, scale=SCALE)
                                    nc.vector.tensor_mul(
                                        pt2[:, r0:r0 + 128],
                                        pt2[:, r0:r0 + 128], msk_sb[:])
                                    nc.vector.tensor_mul(
                                        pt2[:, 512 + r1:512 + r1 + 128],
                                        pt2[:, 512 + r1:512 + r1 + 128],
                                        msk_sb[:])
                                else:                # two full blocks
                                    j0 = 2 * jp
                                    nc.tensor.matmul(
                                        stp2[:, 0:512],
                                        ksb[:, j0 * 128:(j0 + 1) * 128],
                                        qcols, start=True, stop=True)
                                    nc.tensor.matmul(
                                        stp2[:, 512:1024],
                                        ksb[:, (j0 + 1) * 128:
                                            (j0 + 2) * 128],
                                        qcols, start=True, stop=True)
                                    nc.scalar.activation(
                                        pt2[:], stp2[:], AFT.Exp,
                                        scale=SCALE)
                                if pend is not None:
                                    emit_pv(pend)
                                if jp == 1 and prev_acc is not None:
                                    # previous ib's acc closed at jp==0's
                                    # flush; drain it on DVE under this
                                    # ib's matmuls
                                    drain_prev(ib - 1)
                                    prev_acc = None
                                if ib == 1 and jp == 1:
                                    # previous qh's normalization tail,
                                    # now safely off the critical path
                                    flush_norm()
                                pend = (acc, pt2, jp, ib, jp == npair - 1)
                            prev_acc = acc
                        emit_pv(pend)
                        drain_prev(3)
                        norm_tail[0] = (b, qh, dn4, atus)
                flush_norm()

            # ---------------- Phase C: output projection -----------------
            with (
                tc.tile_pool(name="opool", bufs=2) as opool,
                tc.tile_pool(name="cps", bufs=1, space="PSUM") as cps,
            ):
                for ct in range(CO // 128):
                    ops = [cps.tile([128, 512], F32, tag=f"c{tb}",
                                    name=f"ops{ct}_{tb}")
                           for tb in range(T // 512)]
                    for oc in range(OSH // 128):
                        for tb in range(T // 512):
                            nc.tensor.matmul(
                                ops[tb][:],
                                wo_sb[:, oc, ct * 128:(ct + 1) * 128],
                                at_sb[:, oc, tb * 512:(tb + 1) * 512],
                                start=(oc == 0), stop=(oc == OSH // 128 - 1))
                    osb = opool.tile([128, T], BF16, tag="osb",
                                     name=f"osb_{ct}")
                    for tb in range(T // 512):
                        tbs = slice(tb * 512, (tb + 1) * 512)
                        if tb % 3 == 1:
                            nc.scalar.copy(osb[:, tbs], ops[tb][:])
                        else:
                            nc.vector.tensor_copy(osb[:, tbs], ops[tb][:])
                    for dd in range(4):
                        dsl = slice(dd * 1024, (dd + 1) * 1024)
                        eng = nc.gpsimd if dd % 2 == 0 else nc.sync
                        eng.dma_start(
                            out=out[ct * 128:(ct + 1) * 128, dsl],
                            in_=osb[:, dsl])

    nc.compile()
    return nc


_NC_CACHE = None


def _get_nc():
    global _NC_CACHE
    if _NC_CACHE is None:
        _NC_CACHE = _build()
    return _NC_CACHE


def _host_tables(position_ids):
    pos = np.asarray(position_ids).reshape(-1)[:S].astype(np.float64)
    inv_freq = 1.0 / (THETA ** (np.arange(0, ROPE, 2, dtype=np.float64) / ROPE))
    freqs = np.outer(pos, inv_freq)                       # [S, ROPE/2]
    emb = np.concatenate([freqs, freqs], axis=-1)         # [S, ROPE]
    cos_t = np.tile(np.cos(emb).astype(np.float32).T, (1, B))  # [ROPE, T]
    sinf = np.sin(freqs).astype(np.float32).T                  # [ROPE/2, S]
    # rows 0:32 = +sin (t1[32:64] = q[0:32]*sin)
    # rows 32:64 = -sin (t1[0:32] = -q[32:64]*sin)
    sin_t = np.tile(np.concatenate([sinf, -sinf], axis=0), (1, B))
    return cos_t, sin_t


def kernel(hidden_states, attention_mask, position_ids, Wq, Wk, Wv, Wo,
           _trace=False, _tmpdir=None):
    import ml_dtypes
    from concourse.bass_utils import run_bass_kernel_spmd
    bf16 = ml_dtypes.bfloat16

    hidden = np.asarray(hidden_states, dtype=np.float32).reshape(T, HID)
    hiddent = np.ascontiguousarray(hidden.T).astype(bf16)
    Wq = np.asarray(Wq, dtype=np.float32)
    Wk = np.asarray(Wk, dtype=np.float32)
    Wv = np.asarray(Wv, dtype=np.float32)
    Wo = np.asarray(Wo, dtype=np.float32)
    cos_t, sin_t = _host_tables(position_ids)
    m = np.asarray(attention_mask).reshape(S, S)
    tri = np.ascontiguousarray((m[0:128, 0:128] == 0.0).T).astype(bf16)
    ident = np.eye(128, dtype=np.float32).astype(bf16)

    nc = _get_nc()
    in_maps = []
    for c in range(NCORES):
        in_maps.append({
            "hiddent": hiddent,
            "wq": np.ascontiguousarray(
                Wq[:, c * QH * HD:(c + 1) * QH * HD]).astype(bf16),
            "wk": np.ascontiguousarray(Wk[:, c * HD:(c + 1) * HD]).astype(bf16),
            "wv": np.ascontiguousarray(Wv[:, c * VD:(c + 1) * VD]).astype(bf16),
            "wo": np.ascontiguousarray(Wo[c * OSH:(c + 1) * OSH, :]).astype(bf16),
            "cos": cos_t, "sin": sin_t,
            "tri": tri, "ident": ident,
        })
    res = run_bass_kernel_spmd(nc, in_maps, list(range(NCORES)),
                               trace=_trace, tmpdir=_tmpdir)
    full = np.zeros((CO, T), dtype=np.float32)
    for c in range(NCORES):
        full += res.results[c]["out"].astype(np.float32)
    out = np.ascontiguousarray(full.T).reshape(B, S, CO)
    if _trace:
        kernel.last_exec_time_ns = res.exec_time_ns
        kernel.last_profile = res.profile_json
    return out
